# revision 1
# baseline (speedup 1.0000x reference)
"""Trainium2 Bass kernel for GNN mean aggregation (nn_AggrGSMean).

Computes, for t in {0,1}:
    out_t[b, v, :] = segment_sum(features_t over edges with dest v) / degree[b, v, t]
where degree[b, v, t] = max(count(adjacency[b, v, t, :] >= 0), 1).

Strategy (graph-partition sharding per the problem's sharding hint):
- Host: partition edges by destination-vertex range across 8 cores, sort each
  core's edges by destination, group into 128-vertex blocks.  Each block's edge
  list is padded to a whole number of 128-edge tiles.  Blocks are assigned to
  "slots" in decreasing-tile-count order so one static per-slot tile profile
  (max over cores/tables at each rank) serves all cores with ~8% less padding
  than a uniform max.  Features ship as bf16 hi+lo halves (their sum is the
  fp32 value to ~1e-5) plus the destination slot-in-block encoded as a float.
- Device (per core): for each slot, stream 128-edge tiles [hi64|lo64|negv]
  bf16; build a one-hot [128 edges x 128 vslots] in bf16 (iota == vslot) on
  DVE (a fraction on ScalarE via relu(1-(iota-v)^2)); one matmul per tile
  accumulates onehot.T @ [hi|lo] into PSUM [128, 128].  Degree comes from the
  adjacency slice on-chip; the hi/lo halves are summed by a strided
  tensor_reduce and the mean division rides the ScalarE copy (per-partition
  scale = 1/degree).
"""

import sys

if "/opt/trn_rl_repo" not in sys.path:
    sys.path.insert(0, "/opt/trn_rl_repo")

import ml_dtypes
import numpy as np

# Problem constants (hardcoded per contract)
B, V, T, N, F, M = 1, 100000, 2, 32, 64, 1600000
NCORES = 8
BLK = 128           # edges per tile (matmul contraction)
BLK_V = 96          # vertices per block / one-hot width
EW = 2 * F + 2      # bf16 words per edge row: 64 hi | 64 lo | negv f32 (2)
ADJ_G = 7

ONE_F32_U16 = np.array([0x0000, 0x3F80], dtype=np.uint16)  # f32 1.0 as 2 LE u16


class Cfg:
    def __init__(self, v=V, ncores=NCORES):
        self.V = v
        self.NCORES = ncores
        self.VLOC = v // ncores
        nblk = (self.VLOC + BLK_V - 1) // BLK_V
        self.NBLK = ((nblk + ADJ_G - 1) // ADJ_G) * ADJ_G
        self.VPAD = self.NBLK * BLK_V


_DEFAULT_CFG = Cfg()
_NC_CACHE = {}


def build_device_program(
    profile,
    cfg=_DEFAULT_CFG,
    act_frac=0.16,
    gp_frac=0.0,
    dve_chunk=16,
    gp_chunk=16,
):
    """Build + compile the per-core Bass program.

    profile: per-slot tile counts (len NBLK); same static schedule on all cores.
    One-hot builds are batched (k tiles per instruction via broadcast
    tensor_tensor is_equal) and split across DVE / GPSIMD / ScalarE by the
    given fractions to balance engine load.
    """
    from contextlib import ExitStack

    import concourse.tile as tile
    from concourse import bacc, mybir

    f32 = mybir.dt.float32
    bf16 = mybir.dt.bfloat16
    i32 = mybir.dt.int32
    NBLK = cfg.NBLK
    assert len(profile) == NBLK and NBLK % ADJ_G == 0
    t_max = max(profile)
    slot_elems = [BLK * ts * EW for ts in profile]  # edge rows are 128/tile
    slot_base = np.concatenate([[0], np.cumsum(slot_elems)]).astype(np.int64)
    total_elems = int(slot_base[-1])

    nc = bacc.Bacc("TRN2", target_bir_lowering=False, debug=False)
    feat_d = [
        nc.dram_tensor(f"feat{t}", [total_elems], bf16, kind="ExternalInput").ap()
        for t in range(T)
    ]
    adj_d = nc.dram_tensor(
        "adj", [NBLK // ADJ_G, BLK_V, ADJ_G * T * N], i32, kind="ExternalInput"
    ).ap()
    # iota_neg[e, j] = -j (f32) for DVE is_equal against negv;
    # iota_pos[e, j] = +j (bf16) for the ScalarE (j + negv)^2 path
    iota_n_d = nc.dram_tensor("iota_neg", [BLK, BLK_V], f32, kind="ExternalInput").ap()
    iota_p_d = nc.dram_tensor("iota_pos", [BLK, BLK_V], bf16, kind="ExternalInput").ap()
    out_d = nc.dram_tensor("out", [NBLK, BLK_V, T * F], f32, kind="ExternalOutput").ap()

    with tile.TileContext(nc) as tc, ExitStack() as ctx:
        const = ctx.enter_context(tc.tile_pool(name="const", bufs=1))
        featp = ctx.enter_context(tc.tile_pool(name="featp", bufs=6))
        adjp = ctx.enter_context(tc.tile_pool(name="adjp", bufs=3))
        degp = ctx.enter_context(tc.tile_pool(name="degp", bufs=3))
        ohdp = ctx.enter_context(tc.tile_pool(name="ohdp", bufs=6))
        ohgp = ctx.enter_context(tc.tile_pool(name="ohgp", bufs=3))
        ohap = ctx.enter_context(tc.tile_pool(name="ohap", bufs=7))
        redp = ctx.enter_context(tc.tile_pool(name="redp", bufs=5))
        outp = ctx.enter_context(tc.tile_pool(name="outp", bufs=4))
        psump = ctx.enter_context(tc.tile_pool(name="psum", bufs=6, space="PSUM"))

        iota_n = const.tile([BLK, BLK_V], f32)
        nc.sync.dma_start(out=iota_n[:], in_=iota_n_d[:])
        iota_p = const.tile([BLK, BLK_V], bf16)
        nc.sync.dma_start(out=iota_p[:], in_=iota_p_d[:])

        # --- one-hot build planning: weighted chunk assignment across engines ---
        n_tot = [0]
        n_act = [0]
        n_gp = [0]

        def plan_chunks(t_s):
            chunks = []
            i = 0
            while i < t_s:
                rest = t_s - i
                if n_gp[0] + gp_chunk <= gp_frac * (n_tot[0] + gp_chunk) and rest >= gp_chunk:
                    k = gp_chunk
                    chunks.append(("gp", i, k))
                    n_gp[0] += k
                elif n_act[0] < act_frac * n_tot[0]:
                    k = 1
                    chunks.append(("act", i, k))
                    n_act[0] += k
                else:
                    k = min(dve_chunk, rest)
                    chunks.append(("dve", i, k))
                i += k
                n_tot[0] += k
            return chunks

        def build_onehots(feat3, t_s):
            """Build all t_s one-hots for a slot; returns per-tile (tile, col0)."""
            refs = [None] * t_s
            for kind, i0, k in plan_chunks(t_s):
                if kind == "act":
                    negv = feat3[:, i0, 2 * F : 2 * F + 2].bitcast(f32)
                    y = ohap.tile([BLK, BLK_V], bf16, tag="y")
                    nc.scalar.activation(
                        y[:], iota_p[:], mybir.ActivationFunctionType.Square,
                        bias=negv, scale=1.0,
                    )
                    oh = ohap.tile([BLK, BLK_V], bf16, tag="oha")
                    nc.scalar.activation(
                        oh[:], y[:], mybir.ActivationFunctionType.Relu,
                        bias=1.0, scale=-1.0,
                    )
                    refs[i0] = (oh, 0)
                else:
                    eng = nc.gpsimd if kind == "gp" else nc.vector
                    pool_k = gp_chunk if kind == "gp" else dve_chunk
                    pool = ohgp if kind == "gp" else ohdp
                    oh = pool.tile([BLK, pool_k * BLK_V], bf16, tag="oh" + kind)
                    oh3 = oh[:, : k * BLK_V].rearrange("p (i v) -> p i v", v=BLK_V)
                    in0 = iota_n[:].unsqueeze(1).broadcast_to([BLK, k, BLK_V])
                    negv = feat3[:, i0 : i0 + k, 2 * F : 2 * F + 2].bitcast(f32)
                    in1 = negv.broadcast_to([BLK, k, BLK_V])
                    eng.tensor_tensor(oh3, in0, in1, op=mybir.AluOpType.is_equal)
                    for j in range(k):
                        refs[i0 + j] = (oh, j * BLK_V)
            return refs

        for bg in range(NBLK // ADJ_G):
            adj_t = adjp.tile([BLK_V, ADJ_G * T * N], i32)
            nc.sync.dma_start(out=adj_t[:], in_=adj_d[bg])
            val = degp.tile([BLK_V, ADJ_G * T * N], f32, tag="val")
            nc.vector.tensor_scalar(
                val[:], adj_t[:], 0, None, op0=mybir.AluOpType.is_ge
            )
            deg = degp.tile([BLK_V, ADJ_G * T], f32, tag="deg")
            nc.vector.tensor_reduce(
                deg[:],
                val[:].rearrange("p (g n) -> p g n", n=N),
                axis=mybir.AxisListType.X,
                op=mybir.AluOpType.add,
            )
            rec = degp.tile([BLK_V, ADJ_G * T], f32, tag="rec")
            nc.vector.tensor_scalar(
                deg[:], deg[:], 1.0, None, op0=mybir.AluOpType.max
            )
            nc.vector.reciprocal(rec[:], deg[:])

            for bo in range(ADJ_G):
                s = bg * ADJ_G + bo
                t_s = profile[s]
                out_t = outp.tile([BLK_V, T * F], f32)
                for t in range(T):
                    feat_t = featp.tile([BLK, t_max * EW], bf16, tag="feat")
                    src = feat_d[t][
                        int(slot_base[s]) : int(slot_base[s + 1])
                    ].rearrange("(e w) -> e w", w=t_s * EW)
                    nc.sync.dma_start(out=feat_t[:, : t_s * EW], in_=src)
                    feat3 = feat_t[:, : t_s * EW].rearrange(
                        "p (i w) -> p i w", w=EW
                    )
                    oh_refs = build_onehots(feat3, t_s)
                    ps = psump.tile([BLK_V, 2 * F], f32)
                    for i in range(t_s):
                        oh, col0 = oh_refs[i]
                        nc.tensor.matmul(
                            ps[:],
                            lhsT=oh[:, col0 : col0 + BLK_V],
                            rhs=feat_t[:, i * EW : i * EW + 2 * F],
                            start=(i == 0),
                            stop=(i == t_s - 1),
                        )
                    # sum hi+lo halves: [128, (2,64)] -> [128, 64]
                    red = redp.tile([BLK_V, F], f32)
                    nc.vector.tensor_reduce(
                        red[:],
                        ps[:].rearrange("p (h f) -> p f h", h=2),
                        axis=mybir.AxisListType.X,
                        op=mybir.AluOpType.add,
                    )
                    # mean = sum * (1/deg) on ScalarE
                    nc.scalar.mul(
                        out_t[:, t * F : (t + 1) * F],
                        red[:],
                        rec[:, bo * T + t : bo * T + t + 1],
                    )
                nc.sync.dma_start(out=out_d[s], in_=out_t[:])

    nc.compile()
    return nc


def shard_table(indices, cfg=_DEFAULT_CFG):
    """Sort edges by destination and partition by core.

    Returns per-core list of (orig_edge_idx sorted by dest, block, rank_in_block,
    tiles_per_block)."""
    v = np.ascontiguousarray(indices[:, 1])
    order = np.argsort(v, kind="stable")
    vs = v[order]
    bounds = np.searchsorted(vs, np.arange(cfg.NCORES + 1) * cfg.VLOC)
    per_core = []
    for c in range(cfg.NCORES):
        lo, hi = bounds[c], bounds[c + 1]
        idx = order[lo:hi]
        vloc = vs[lo:hi].astype(np.int64) - c * cfg.VLOC
        blk = vloc // BLK_V
        vin = vloc % BLK_V
        cnt = np.bincount(blk, minlength=cfg.NBLK).astype(np.int64)
        starts = np.zeros(cfg.NBLK, dtype=np.int64)
        np.cumsum(cnt[:-1], out=starts[1:])
        rank = np.arange(len(idx), dtype=np.int64) - starts[blk]
        tiles = (cnt + BLK - 1) // BLK
        per_core.append((idx, blk, vin, rank, tiles))
    return per_core


def make_profile(per_core_tables, cfg=_DEFAULT_CFG):
    """Slot tile profile + per (core, table) block->slot permutation."""
    perms = []  # perms[t][c] = array: slot -> block
    sorted_tiles = []
    for per_core in per_core_tables:
        perms_t = []
        for c in range(cfg.NCORES):
            tiles = per_core[c][4]
            order = np.argsort(-tiles, kind="stable")
            perms_t.append(order)
            sorted_tiles.append(tiles[order])
        perms.append(perms_t)
    profile = np.max(np.stack(sorted_tiles), axis=0)
    profile = np.maximum(profile, 1)
    return [int(x) for x in profile], perms


def fill_feature_stream(per_core, features, profile, perm_t, cfg=_DEFAULT_CFG):
    """Per-core bf16 edge stream, slot-major, edge-slot-major within a slot.

    Row layout (130 bf16 words): [hi(64) | lo(64) | negv as f32 (2 words)].
    Padding rows have negv = +1.0 (never matches iota_neg <= 0)."""
    prof = np.asarray(profile, dtype=np.int64)
    row_base = np.concatenate([[0], np.cumsum(prof * BLK)]).astype(np.int64)
    total_rows = int(row_base[-1])

    hi = features.astype(ml_dtypes.bfloat16)
    lo = (features - hi.astype(np.float32)).astype(ml_dtypes.bfloat16)
    hi_u = hi.view(np.uint16)
    lo_u = lo.view(np.uint16)

    out = np.zeros((cfg.NCORES, total_rows, EW), dtype=np.uint16)
    out[:, :, 2 * F : 2 * F + 2] = ONE_F32_U16  # negv = +1.0 for padding rows
    for c in range(cfg.NCORES):
        idx, blk, vin, rank, _tiles = per_core[c]
        inv = np.empty(cfg.NBLK, dtype=np.int64)
        inv[perm_t[c]] = np.arange(cfg.NBLK)
        s = inv[blk]
        rows = row_base[s] + (rank & 127) * prof[s] + (rank >> 7)
        out[c, rows, 0:F] = hi_u[idx]
        out[c, rows, F : 2 * F] = lo_u[idx]
        out[c, rows, 2 * F : 2 * F + 2] = (
            (-vin.astype(np.float32)).view(np.uint32).view(np.uint16).reshape(-1, 2)
        )
    return out.reshape(cfg.NCORES, total_rows * EW).view(ml_dtypes.bfloat16)


def prep_adjacency(adjacency, perms, cfg=_DEFAULT_CFG):
    """adj_dev[c, g, vin, j*64 + t*32 + n] = adjacency[0, block_{t}(c, 7g+j), vin, t, n]
    padded with -1 beyond VLOC."""
    adj = np.ascontiguousarray(adjacency.reshape(cfg.V, T, N))
    adj_pad = np.full((cfg.NCORES, cfg.VPAD, T, N), -1, dtype=np.int32)
    adj_pad[:, : cfg.VLOC] = adj.reshape(cfg.NCORES, cfg.VLOC, T, N)
    adj_pad = adj_pad.reshape(cfg.NCORES, cfg.NBLK, BLK_V, T, N)
    out = np.empty((cfg.NCORES, cfg.NBLK, BLK_V, T, N), dtype=np.int32)
    for c in range(cfg.NCORES):
        for t in range(T):
            out[c, :, :, t, :] = adj_pad[c, perms[t][c], :, t, :]
    # [c, g, j, vin, t, n] -> [c, g, vin, j, t, n]
    out = out.reshape(cfg.NCORES, cfg.NBLK // ADJ_G, ADJ_G, BLK_V, T * N)
    out = np.ascontiguousarray(out.transpose(0, 1, 3, 2, 4))
    return out.reshape(cfg.NCORES, cfg.NBLK // ADJ_G, BLK_V, ADJ_G * T * N)


def prepare_inputs(adjacency, indices0, features0, indices1, features1, cfg=_DEFAULT_CFG):
    adjacency = np.asarray(adjacency)
    pc0 = shard_table(np.asarray(indices0), cfg)
    pc1 = shard_table(np.asarray(indices1), cfg)
    profile, perms = make_profile([pc0, pc1], cfg)

    f0 = fill_feature_stream(
        pc0, np.asarray(features0, dtype=np.float32), profile, perms[0], cfg
    )
    f1 = fill_feature_stream(
        pc1, np.asarray(features1, dtype=np.float32), profile, perms[1], cfg
    )
    adj = prep_adjacency(adjacency, perms, cfg)
    iota_neg = np.broadcast_to(
        -np.arange(BLK_V, dtype=np.float32), (BLK, BLK_V)
    ).copy()
    iota_pos = np.broadcast_to(
        np.arange(BLK_V).astype(ml_dtypes.bfloat16), (BLK, BLK_V)
    ).copy()

    in_maps = [
        {
            "feat0": f0[c],
            "feat1": f1[c],
            "adj": adj[c],
            "iota_neg": iota_neg,
            "iota_pos": iota_pos,
        }
        for c in range(cfg.NCORES)
    ]
    return in_maps, profile, perms


def assemble_output(core_outs, perms, cfg=_DEFAULT_CFG):
    outs = []
    for t in range(T):
        parts = []
        for c in range(cfg.NCORES):
            res_t = core_outs[c].reshape(cfg.NBLK, BLK_V, T, F)[:, :, t, :]
            tmp = np.empty((cfg.NBLK, BLK_V, F), dtype=res_t.dtype)
            tmp[perms[t][c]] = res_t
            parts.append(tmp.reshape(cfg.VPAD, F)[: cfg.VLOC])
        outs.append(np.concatenate(parts, axis=0).reshape(B, cfg.V, F))
    return (outs[0], outs[1])


def kernel(adjacency, indices0, features0, indices1, features1):
    from concourse.bass_utils import run_bass_kernel_spmd

    cfg = _DEFAULT_CFG
    in_maps, profile, perms = prepare_inputs(
        adjacency, indices0, features0, indices1, features1, cfg
    )

    key = tuple(profile)
    if key not in _NC_CACHE:
        _NC_CACHE[key] = build_device_program(profile, cfg)
    nc = _NC_CACHE[key]

    res = run_bass_kernel_spmd(nc, in_maps, list(range(cfg.NCORES)))
    return assemble_output(
        [res.results[c]["out"] for c in range(cfg.NCORES)], perms, cfg
    )



# revision 3
# speedup vs baseline: 2.4163x; 2.4163x over previous
"""Trainium2 Bass kernel for GNN mean aggregation (nn_AggrGSMean).

Computes, for t in {0,1}:
    out_t[b, v, :] = segment_sum(features_t over edges with dest v) / degree[b, v, t]
where degree[b, v, t] = max(count(adjacency[b, v, t, :] >= 0), 1).

Strategy (graph-partition sharding):
- Host: partition edges by destination-vertex range across 8 cores, sort each
  core's edges by destination.  Vertices form 128-wide blocks (the PSUM
  partition dim), each split into four 32-vertex windows.  Each window's edge
  list is padded to whole 128-edge tiles.  Windows are sorted within a block
  and blocks sorted by tile count so one static profile (max over cores and
  tables at each rank) serves all cores.  Features ship as single bf16 (the
  2e-2 rel-err budget gives ~70x margin); each edge also carries an int16
  "flat one-hot index" = tile_in_slot*32 + vertex_in_window.
- The HBM stream is partition-major per 7-slot group, so each DMA moves
  [128, ~16KB] with fully contiguous lines (the baseline was packet-rate
  bound at ~2KB lines).  Table 0 streams on the sync-engine HWDGE queue,
  table 1 on the scalar-engine queue.
- Device per (slot, table): one DVE is_equal builds all one-hots at once
  (iota_int16 vs broadcast idx -> bf16 [128, T_s*32]); per window a chain of
  matmuls with 32-column stationary one-hots accumulates into a 32-partition
  slice of a [128, 64] PSUM tile (tile_position from out.base_partition).
  Degree comes from an int8 adjacency slice (is_ge + reduce + recip on DVE);
  the mean division rides a ScalarE copy into a grouped output tile.
"""

import sys

if "/opt/trn_rl_repo" not in sys.path:
    sys.path.insert(0, "/opt/trn_rl_repo")

import ml_dtypes
import numpy as np

# Problem constants (hardcoded per contract)
B, V, T, N, F, M = 1, 100000, 2, 32, 64, 1600000
NCORES = 8
BLK = 128            # edges per tile (matmul contraction)
BLK_V = 128          # vertices per block (PSUM partition dim)
WIN = 32             # vertices per one-hot window (stationary columns)
NW = BLK_V // WIN    # windows per block
GRP = 7              # slots per DMA group


class Cfg:
    def __init__(self, v=V, ncores=NCORES):
        self.V = v
        self.NCORES = ncores
        self.VLOC = v // ncores
        nblk = (self.VLOC + BLK_V - 1) // BLK_V
        self.NBLK = ((nblk + GRP - 1) // GRP) * GRP
        self.NG = self.NBLK // GRP
        self.VPAD = self.NBLK * BLK_V


_DEFAULT_CFG = Cfg()
_NC_CACHE = {}


def _layout(profile):
    """Shared host/device derived layout from the [NBLK, 4] tile profile."""
    profile = np.asarray(profile, dtype=np.int64)
    nblk = profile.shape[0]
    prof_t = profile.sum(axis=1)                      # tiles per slot
    fo = np.zeros((nblk, NW), dtype=np.int64)         # flat tile offset per window rank
    fo[:, 1:] = np.cumsum(profile[:, :-1], axis=1)
    iw = (prof_t + 1) // 2 * 2                        # idx cols per slot (even)
    ng = nblk // GRP
    pt = prof_t.reshape(ng, GRP)
    iwg = iw.reshape(ng, GRP)
    fb = np.zeros((ng, GRP), dtype=np.int64)          # feat word offset of slot in group line
    fb[:, 1:] = np.cumsum(pt[:, :-1] * (2 * F // 2), axis=1)  # 64 words per tile row
    fw = (pt * F).sum(axis=1)                         # feat words per group line (64*prof_t)
    ib = np.zeros((ng, GRP), dtype=np.int64)
    ib[:, 1:] = np.cumsum(iwg[:, :-1], axis=1)
    lw = fw + iwg.sum(axis=1)                         # total words per partition line
    gb = np.zeros(ng + 1, dtype=np.int64)             # word offset of group in stream
    gb[1:] = np.cumsum(lw * BLK)
    return dict(profile=profile, prof_t=prof_t, fo=fo, iw=iw, fb=fb, fw=fw,
                ib=ib, lw=lw, gb=gb, tmax=int(prof_t.max()))


def build_device_program(profile, cfg=_DEFAULT_CFG):
    """Build + compile the per-core Bass program for a [NBLK, 4] profile."""
    from contextlib import ExitStack

    import concourse.tile as tile
    from concourse import bacc, mybir

    f32 = mybir.dt.float32
    bf16 = mybir.dt.bfloat16
    i16 = mybir.dt.int16
    i8 = mybir.dt.int8

    lay = _layout(profile)
    prof = lay["profile"]
    prof_t, fo, fb, fw, ib, lw, gb = (lay[k] for k in
                                      ("prof_t", "fo", "fb", "fw", "ib", "lw", "gb"))
    NBLK, NG = cfg.NBLK, cfg.NG
    tmax = lay["tmax"]
    lwmax = int(lw.max())

    nc = bacc.Bacc("TRN2", target_bir_lowering=False, debug=False)
    feat_d = [
        nc.dram_tensor(f"feat{t}", [int(gb[-1])], bf16, kind="ExternalInput").ap()
        for t in range(T)
    ]
    adj_d = nc.dram_tensor("adj", [NG, BLK, GRP * T * N], i8, kind="ExternalInput").ap()
    iota_d = nc.dram_tensor("iota", [BLK, tmax * WIN], i16, kind="ExternalInput").ap()
    out_d = nc.dram_tensor("out", [NG, BLK, GRP * T * F], f32, kind="ExternalOutput").ap()

    with tile.TileContext(nc) as tc, ExitStack() as ctx:
        const = ctx.enter_context(tc.tile_pool(name="const", bufs=1))
        featp = ctx.enter_context(tc.tile_pool(name="featp", bufs=4))
        adjp = ctx.enter_context(tc.tile_pool(name="adjp", bufs=3))
        degp = ctx.enter_context(tc.tile_pool(name="degp", bufs=3))
        ohp = ctx.enter_context(tc.tile_pool(name="ohp", bufs=6))
        outp = ctx.enter_context(tc.tile_pool(name="outp", bufs=3))
        psump = ctx.enter_context(tc.tile_pool(name="psum", bufs=8, space="PSUM"))

        iota = const.tile([BLK, tmax * WIN], i16)
        nc.sync.dma_start(out=iota[:], in_=iota_d[:])

        for g in range(NG):
            adj_t = adjp.tile([BLK, GRP * T * N], i8)
            nc.sync.dma_start(out=adj_t[:], in_=adj_d[g])
            val = degp.tile([BLK, GRP * T * N], bf16, tag="val")
            nc.vector.tensor_scalar(
                val[:], adj_t[:], 0, None, op0=mybir.AluOpType.is_ge
            )
            deg = degp.tile([BLK, GRP * T], f32, tag="deg")
            nc.vector.tensor_reduce(
                deg[:],
                val[:].rearrange("p (x n) -> p x n", n=N),
                axis=mybir.AxisListType.X,
                op=mybir.AluOpType.add,
            )
            rec = degp.tile([BLK, GRP * T], f32, tag="rec")
            nc.vector.tensor_scalar(
                deg[:], deg[:], 1.0, None, op0=mybir.AluOpType.max
            )
            nc.vector.reciprocal(rec[:], deg[:])

            feats = []
            for t in range(T):
                ft = featp.tile([BLK, lwmax], bf16, tag=f"feat{t}")
                src = feat_d[t][int(gb[g]) : int(gb[g + 1])].rearrange(
                    "(p w) -> p w", w=int(lw[g])
                )
                eng = nc.sync if t == 0 else nc.scalar
                eng.dma_start(out=ft[:, : int(lw[g])], in_=src)
                feats.append(ft)

            out_t = outp.tile([BLK, GRP * T * F], f32)
            for q in range(GRP):
                s = g * GRP + q
                ts = int(prof_t[s])
                for t in range(T):
                    ft = feats[t]
                    fbase = int(fb[g, q])
                    idx = ft[:, int(fw[g]) + int(ib[g, q]) :
                             int(fw[g]) + int(ib[g, q]) + ts].bitcast(i16)
                    oh = ohp.tile([BLK, tmax * WIN], bf16, tag="oh")
                    nc.vector.tensor_tensor(
                        oh[:, : ts * WIN].rearrange("p (i v) -> p i v", v=WIN),
                        iota[:, : ts * WIN].rearrange("p (i v) -> p i v", v=WIN),
                        idx.unsqueeze(2).broadcast_to([BLK, ts, WIN]),
                        op=mybir.AluOpType.is_equal,
                    )
                    ps = psump.tile([BLK, F], f32)
                    for j in range(NW):
                        nt = int(prof[s, j])
                        for il in range(nt):
                            i = int(fo[s, j]) + il
                            nc.tensor.matmul(
                                ps[j * WIN : (j + 1) * WIN, :],
                                lhsT=oh[:, i * WIN : (i + 1) * WIN],
                                rhs=ft[:, fbase + i * F : fbase + (i + 1) * F],
                                start=(il == 0),
                                stop=(il == nt - 1),
                                tile_position=(0, j * WIN),
                            )
                    nc.scalar.mul(
                        out_t[:, (q * T + t) * F : (q * T + t + 1) * F],
                        ps[:],
                        rec[:, q * T + t : q * T + t + 1],
                    )
            nc.sync.dma_start(out=out_d[g], in_=out_t[:])

    nc.compile()
    return nc


def shard_table(indices, cfg=_DEFAULT_CFG):
    """Sort edges by destination, partition by core, build per-core schedule."""
    v = np.ascontiguousarray(indices[:, 1])
    order = np.argsort(v, kind="stable")
    vs = v[order]
    bounds = np.searchsorted(vs, np.arange(cfg.NCORES + 1) * cfg.VLOC)
    per_core = []
    for c in range(cfg.NCORES):
        lo, hi = bounds[c], bounds[c + 1]
        idx_e = order[lo:hi]
        vloc = vs[lo:hi].astype(np.int64) - c * cfg.VLOC
        bw = vloc >> 5                       # block*4 + window
        u = vloc & 31
        cnt = np.bincount(bw, minlength=cfg.NBLK * NW).reshape(cfg.NBLK, NW)
        tiles = (cnt + BLK - 1) // BLK
        win_perm = np.argsort(-tiles, axis=1, kind="stable")   # [NBLK, 4] rank->win
        blk_tot = tiles.sum(axis=1)
        blk_perm = np.argsort(-blk_tot, kind="stable")         # slot->block
        st = np.take_along_axis(tiles, win_perm, axis=1)[blk_perm]
        per_core.append(dict(idx_e=idx_e, bw=bw, u=u, cnt=cnt,
                             win_perm=win_perm, blk_perm=blk_perm, st=st))
    return per_core


def make_profile(tables, cfg=_DEFAULT_CFG):
    """profile[s, j] = max tile count over (core, table) at rank (s, j)."""
    st = np.stack([pc["st"] for per_core in tables for pc in per_core])
    return np.maximum(st.max(axis=0), 1)


def _vert_rows(pc, cfg):
    """vert[s, vin]: local vertex id at psum row vin of slot s (may be >= VLOC)."""
    blk_perm, win_perm = pc["blk_perm"], pc["win_perm"]
    w = win_perm[blk_perm]                                   # [NBLK, 4] rank->win
    vin_off = (w[:, :, None] * WIN + np.arange(WIN)).reshape(cfg.NBLK, BLK_V)
    return blk_perm[:, None] * BLK_V + vin_off


def fill_stream(pc, features, lay, cfg=_DEFAULT_CFG):
    """Per-core bf16+int16 stream, partition-major per 7-slot group."""
    profile, prof_t, fo, iw = lay["profile"], lay["prof_t"], lay["fo"], lay["iw"]
    fb, fw, ib, lw, gb = lay["fb"], lay["fw"], lay["ib"], lay["lw"], lay["gb"]

    blk_perm, win_perm = pc["blk_perm"], pc["win_perm"]
    inv_blk = np.empty(cfg.NBLK, dtype=np.int64)
    inv_blk[blk_perm] = np.arange(cfg.NBLK)
    winrank = np.empty((cfg.NBLK, NW), dtype=np.int64)
    np.put_along_axis(winrank, win_perm,
                      np.broadcast_to(np.arange(NW), (cfg.NBLK, NW)), axis=1)

    bw, u, cnt = pc["bw"], pc["u"], pc["cnt"]
    starts = np.zeros(cfg.NBLK * NW, dtype=np.int64)
    np.cumsum(cnt.ravel()[:-1], out=starts[1:])
    r = np.arange(len(bw), dtype=np.int64) - starts[bw]
    b = bw >> 2
    s = inv_blk[b]
    j = winrank[b, bw & 3]
    i_flat = fo[s, j] + (r >> 7)
    p = r & 127

    hi = features.astype(ml_dtypes.bfloat16).view(np.uint16)

    # slot-major row store: row id = RB[s] + i_flat*128 + p
    rb = np.zeros(cfg.NBLK + 1, dtype=np.int64)
    rb[1:] = np.cumsum(prof_t * BLK)
    rows = np.zeros((int(rb[-1]), F), dtype=np.uint16)
    rows[rb[s] + (i_flat << 7) + p] = hi[pc["idx_e"]]

    # idx store: [prof_t, 128] per slot, default i*32 (pad -> window col 0)
    ivb = np.zeros(cfg.NBLK + 1, dtype=np.int64)
    ivb[1:] = np.cumsum(prof_t * BLK)
    ival = np.concatenate(
        [np.repeat(np.arange(prof_t[ss], dtype=np.int16) * WIN, BLK)
         for ss in range(cfg.NBLK)]
    )
    ival[ivb[s] + (i_flat << 7) + p] = (i_flat * WIN + u).astype(np.int16)
    ival_u = ival.view(np.uint16)

    stream = np.empty(int(gb[-1]), dtype=np.uint16)
    for g in range(cfg.NG):
        vg = stream[int(gb[g]) : int(gb[g + 1])].reshape(BLK, int(lw[g]))
        for q in range(GRP):
            ss = g * GRP + q
            pt = int(prof_t[ss])
            blkrows = rows[rb[ss] : rb[ss + 1]].reshape(pt, BLK, F)
            vg[:, int(fb[g, q]) : int(fb[g, q]) + pt * F] = (
                blkrows.transpose(1, 0, 2).reshape(BLK, pt * F)
            )
            iarr = ival_u[ivb[ss] : ivb[ss + 1]].reshape(pt, BLK).T
            o = int(fw[g]) + int(ib[g, q])
            vg[:, o : o + pt] = iarr
            if int(iw[ss]) > pt:
                vg[:, o + pt : o + int(iw[ss])] = 0xFFFF  # int16 -1 pad col
    return stream.view(ml_dtypes.bfloat16)


def prep_adjacency(adjacency, pcs, cfg=_DEFAULT_CFG):
    """adj8[c][g, vin, q*T*N + t*N + n] for the permuted vertex at (slot, vin)."""
    adj = np.ascontiguousarray(adjacency.reshape(cfg.V, T, N)).astype(np.int8)
    outs = []
    for c in range(cfg.NCORES):
        apad = np.full((cfg.VPAD, T, N), -1, dtype=np.int8)
        lo = c * cfg.VLOC
        apad[: cfg.VLOC] = adj[lo : lo + cfg.VLOC]
        dev = np.empty((cfg.NBLK, BLK_V, T, N), dtype=np.int8)
        for t in range(T):
            vert = _vert_rows(pcs[t][c], cfg)           # [NBLK, 128]
            dev[:, :, t, :] = apad[vert, t, :]
        dev = dev.reshape(cfg.NG, GRP, BLK_V, T * N).transpose(0, 2, 1, 3)
        outs.append(np.ascontiguousarray(dev).reshape(cfg.NG, BLK, GRP * T * N))
    return outs


def prepare_inputs(adjacency, indices0, features0, indices1, features1,
                   cfg=_DEFAULT_CFG):
    adjacency = np.asarray(adjacency)
    pcs = [shard_table(np.asarray(indices0), cfg),
           shard_table(np.asarray(indices1), cfg)]
    profile = make_profile(pcs, cfg)
    lay = _layout(profile)

    feats = [np.asarray(features0, dtype=np.float32),
             np.asarray(features1, dtype=np.float32)]
    adj8 = prep_adjacency(adjacency, pcs, cfg)
    iota = np.broadcast_to(
        np.arange(lay["tmax"] * WIN, dtype=np.int16), (BLK, lay["tmax"] * WIN)
    ).copy()

    in_maps = []
    for c in range(cfg.NCORES):
        m = {"adj": adj8[c], "iota": iota}
        for t in range(T):
            m[f"feat{t}"] = fill_stream(pcs[t][c], feats[t], lay, cfg)
        in_maps.append(m)
    return in_maps, profile, pcs


def assemble_output(core_outs, pcs, cfg=_DEFAULT_CFG):
    outs = []
    for t in range(T):
        parts = []
        for c in range(cfg.NCORES):
            res = core_outs[c].reshape(cfg.NG, BLK, GRP, T, F)
            sres = res.transpose(0, 2, 1, 3, 4).reshape(cfg.NBLK, BLK, T, F)
            vert = _vert_rows(pcs[t][c], cfg)
            full = np.empty((cfg.VPAD, F), dtype=np.float32)
            full[vert.ravel()] = sres[:, :, t, :].reshape(-1, F)
            parts.append(full[: cfg.VLOC])
        outs.append(np.concatenate(parts, axis=0).reshape(B, cfg.V, F))
    return (outs[0], outs[1])


def kernel(adjacency, indices0, features0, indices1, features1):
    from concourse.bass_utils import run_bass_kernel_spmd

    cfg = _DEFAULT_CFG
    in_maps, profile, pcs = prepare_inputs(
        adjacency, indices0, features0, indices1, features1, cfg
    )

    key = profile.tobytes()
    if key not in _NC_CACHE:
        _NC_CACHE[key] = build_device_program(profile, cfg)
    nc = _NC_CACHE[key]

    res = run_bass_kernel_spmd(nc, in_maps, list(range(cfg.NCORES)))
    return assemble_output(
        [res.results[c]["out"] for c in range(cfg.NCORES)], pcs, cfg
    )


# revision 12
# speedup vs baseline: 2.4271x; 1.0045x over previous
"""Trainium2 Bass kernel for GNN mean aggregation (nn_AggrGSMean).

Computes, for t in {0,1}:
    out_t[b, v, :] = segment_sum(features_t over edges with dest v) / degree[b, v, t]
where degree[b, v, t] = max(count(adjacency[b, v, t, :] >= 0), 1).

Strategy (graph-partition sharding):
- Host: partition edges by destination-vertex range across 8 cores, sort each
  core's edges by destination.  Vertices form 128-wide blocks (the PSUM
  partition dim), each split into four 32-vertex windows.  Each window's edge
  list is padded to whole 128-edge tiles.  Windows are sorted within a block
  and blocks sorted by tile count so one static profile (max over cores and
  tables at each rank) serves all cores.  Features ship as single bf16 (the
  2e-2 rel-err budget gives ~70x margin); each edge also carries an int16
  "flat one-hot index" = tile_in_slot*32 + vertex_in_window.
- The HBM stream is partition-major per 7-slot group, so each DMA moves
  [128, ~16KB] with fully contiguous lines (the baseline was packet-rate
  bound at ~2KB lines).  Table 0 streams on the sync-engine HWDGE queue,
  table 1 on the scalar-engine queue.
- Device per (slot, table): one DVE is_equal builds all one-hots at once
  (iota_int16 vs broadcast idx -> bf16 [128, T_s*32]); per window a chain of
  matmuls with 32-column stationary one-hots accumulates into a 32-partition
  slice of a [128, 64] PSUM tile (tile_position from out.base_partition).
  Degree comes from an int8 adjacency slice (is_ge + reduce + recip on DVE);
  the mean division rides a ScalarE copy into a grouped output tile.
"""

import sys

if "/opt/trn_rl_repo" not in sys.path:
    sys.path.insert(0, "/opt/trn_rl_repo")

import ml_dtypes
import numpy as np

# Problem constants (hardcoded per contract)
B, V, T, N, F, M = 1, 100000, 2, 32, 64, 1600000
NCORES = 8
BLK = 128            # edges per tile (matmul contraction)
BLK_V = 128          # vertices per block (PSUM partition dim)
WIN = 32             # vertices per one-hot window (stationary columns)
NW = BLK_V // WIN    # windows per block
GRP = 7              # slots per DMA group


class Cfg:
    def __init__(self, v=V, ncores=NCORES):
        self.V = v
        self.NCORES = ncores
        self.VLOC = v // ncores
        nblk = (self.VLOC + BLK_V - 1) // BLK_V
        self.NBLK = ((nblk + GRP - 1) // GRP) * GRP
        self.NG = self.NBLK // GRP
        self.VPAD = self.NBLK * BLK_V


_DEFAULT_CFG = Cfg()
_NC_CACHE = {}


def _layout(profile):
    """Shared host/device derived layout from the [NBLK, 4] tile profile."""
    profile = np.asarray(profile, dtype=np.int64)
    nblk = profile.shape[0]
    prof_t = profile.sum(axis=1)                      # tiles per slot
    fo = np.zeros((nblk, NW), dtype=np.int64)         # flat tile offset per window rank
    fo[:, 1:] = np.cumsum(profile[:, :-1], axis=1)
    iw = (prof_t + 1) // 2 * 2                        # idx cols per slot (even)
    ng = nblk // GRP
    pt = prof_t.reshape(ng, GRP)
    iwg = iw.reshape(ng, GRP)
    fb = np.zeros((ng, GRP), dtype=np.int64)          # feat word offset of slot in group line
    fb[:, 1:] = np.cumsum(pt[:, :-1] * (2 * F // 2), axis=1)  # 64 words per tile row
    fw = (pt * F).sum(axis=1)                         # feat words per group line (64*prof_t)
    ib = np.zeros((ng, GRP), dtype=np.int64)
    ib[:, 1:] = np.cumsum(iwg[:, :-1], axis=1)
    lw = fw + iwg.sum(axis=1)                         # total words per partition line
    gb = np.zeros(ng + 1, dtype=np.int64)             # word offset of group in stream
    gb[1:] = np.cumsum(lw * BLK)
    return dict(profile=profile, prof_t=prof_t, fo=fo, iw=iw, fb=fb, fw=fw,
                ib=ib, lw=lw, gb=gb, tmax=int(prof_t.max()))


def build_device_program(profile, cfg=_DEFAULT_CFG):
    """Build + compile the per-core Bass program for a [NBLK, 4] profile."""
    from contextlib import ExitStack

    import concourse.tile as tile
    from concourse import bacc, mybir

    f32 = mybir.dt.float32
    bf16 = mybir.dt.bfloat16
    i8 = mybir.dt.int8

    lay = _layout(profile)
    prof = lay["profile"]
    prof_t, fo, fb, fw, ib, lw, gb = (lay[k] for k in
                                      ("prof_t", "fo", "fb", "fw", "ib", "lw", "gb"))
    NBLK, NG = cfg.NBLK, cfg.NG
    tmax = lay["tmax"]
    lwmax = int(lw.max())

    nc = bacc.Bacc("TRN2", target_bir_lowering=False, debug=False)
    feat_d = [
        nc.dram_tensor(f"feat{t}", [int(gb[-1])], bf16, kind="ExternalInput").ap()
        for t in range(T)
    ]
    adj_d = nc.dram_tensor("adj", [NG, BLK, GRP * T * N], i8, kind="ExternalInput").ap()
    iota_d = nc.dram_tensor("iota", [BLK, tmax * WIN], bf16, kind="ExternalInput").ap()
    out_d = nc.dram_tensor("out", [NG, BLK, GRP * T * F], bf16, kind="ExternalOutput").ap()

    with tile.TileContext(nc) as tc, ExitStack() as ctx:
        const = ctx.enter_context(tc.tile_pool(name="const", bufs=1))
        featp = ctx.enter_context(tc.tile_pool(name="featp", bufs=4))
        adjp = ctx.enter_context(tc.tile_pool(name="adjp", bufs=3))
        degp = ctx.enter_context(tc.tile_pool(name="degp", bufs=3))
        ohp = ctx.enter_context(tc.tile_pool(name="ohp", bufs=6))
        outp = ctx.enter_context(tc.tile_pool(name="outp", bufs=3))
        psump = ctx.enter_context(tc.tile_pool(name="psum", bufs=8, space="PSUM"))

        iota = const.tile([BLK, tmax * WIN], bf16)
        nc.sync.dma_start(out=iota[:], in_=iota_d[:])

        for g in range(NG):
            adj_t = adjp.tile([BLK, GRP * T * N], i8)
            nc.sync.dma_start(out=adj_t[:], in_=adj_d[g])
            val = degp.tile([BLK, GRP * T * N], bf16, tag="val")
            nc.vector.tensor_scalar(
                val[:], adj_t[:], 0, None, op0=mybir.AluOpType.is_ge
            )
            deg = degp.tile([BLK, GRP * T], f32, tag="deg")
            nc.vector.tensor_reduce(
                deg[:],
                val[:].rearrange("p (x n) -> p x n", n=N),
                axis=mybir.AxisListType.X,
                op=mybir.AluOpType.add,
            )
            rec = degp.tile([BLK, GRP * T], f32, tag="rec")
            nc.vector.tensor_scalar(
                deg[:], deg[:], 1.0, None, op0=mybir.AluOpType.max
            )
            nc.vector.reciprocal(rec[:], deg[:])

            feats = []
            for t in range(T):
                ft = featp.tile([BLK, lwmax], bf16, tag=f"feat{t}")
                src = feat_d[t][int(gb[g]) : int(gb[g + 1])].rearrange(
                    "(p w) -> p w", w=int(lw[g])
                )
                eng = nc.sync if t == 0 else nc.scalar
                eng.dma_start(out=ft[:, : int(lw[g])], in_=src)
                feats.append(ft)

            out_t = outp.tile([BLK, GRP * T * F], bf16)
            for q in range(GRP):
                s = g * GRP + q
                ts = int(prof_t[s])
                for t in range(T):
                    ft = feats[t]
                    fbase = int(fb[g, q])
                    idx = ft[:, int(fw[g]) + int(ib[g, q]) :
                             int(fw[g]) + int(ib[g, q]) + ts]
                    oh = ohp.tile([BLK, tmax * WIN], bf16, tag="oh")
                    nc.vector.tensor_tensor(
                        oh[:, : ts * WIN].rearrange("p (i v) -> p i v", v=WIN),
                        iota[:, : ts * WIN].rearrange("p (i v) -> p i v", v=WIN),
                        idx.unsqueeze(2).broadcast_to([BLK, ts, WIN]),
                        op=mybir.AluOpType.is_equal,
                    )
                    ps = psump.tile([BLK, F], f32)
                    for j in range(NW):
                        nt = int(prof[s, j])
                        for il in range(nt):
                            i = int(fo[s, j]) + il
                            nc.tensor.matmul(
                                ps[j * WIN : (j + 1) * WIN, :],
                                lhsT=oh[:, i * WIN : (i + 1) * WIN],
                                rhs=ft[:, fbase + i * F : fbase + (i + 1) * F],
                                start=(il == 0),
                                stop=(il == nt - 1),
                                tile_position=(0, j * WIN),
                            )
                    nc.scalar.mul(
                        out_t[:, (q * T + t) * F : (q * T + t + 1) * F],
                        ps[:],
                        rec[:, q * T + t : q * T + t + 1],
                    )
            nc.sync.dma_start(out=out_d[g], in_=out_t[:])

    nc.compile()
    return nc


def shard_table(indices, cfg=_DEFAULT_CFG):
    """Sort edges by destination, partition by core, build per-core schedule."""
    v = np.ascontiguousarray(indices[:, 1])
    order = np.argsort(v, kind="stable")
    vs = v[order]
    bounds = np.searchsorted(vs, np.arange(cfg.NCORES + 1) * cfg.VLOC)
    per_core = []
    for c in range(cfg.NCORES):
        lo, hi = bounds[c], bounds[c + 1]
        idx_e = order[lo:hi]
        vloc = vs[lo:hi].astype(np.int64) - c * cfg.VLOC
        bw = vloc >> 5                       # block*4 + window
        u = vloc & 31
        cnt = np.bincount(bw, minlength=cfg.NBLK * NW).reshape(cfg.NBLK, NW)
        tiles = (cnt + BLK - 1) // BLK
        win_perm = np.argsort(-tiles, axis=1, kind="stable")   # [NBLK, 4] rank->win
        blk_tot = tiles.sum(axis=1)
        blk_perm = np.argsort(-blk_tot, kind="stable")         # slot->block
        st = np.take_along_axis(tiles, win_perm, axis=1)[blk_perm]
        per_core.append(dict(idx_e=idx_e, bw=bw, u=u, cnt=cnt,
                             win_perm=win_perm, blk_perm=blk_perm, st=st))
    return per_core


def make_profile(tables, cfg=_DEFAULT_CFG):
    """profile[s, j] = max tile count over (core, table) at rank (s, j)."""
    st = np.stack([pc["st"] for per_core in tables for pc in per_core])
    return np.maximum(st.max(axis=0), 1)


def _vert_rows(pc, cfg):
    """vert[s, vin]: local vertex id at psum row vin of slot s (may be >= VLOC)."""
    blk_perm, win_perm = pc["blk_perm"], pc["win_perm"]
    w = win_perm[blk_perm]                                   # [NBLK, 4] rank->win
    vin_off = (w[:, :, None] * WIN + np.arange(WIN)).reshape(cfg.NBLK, BLK_V)
    return blk_perm[:, None] * BLK_V + vin_off


def fill_stream(pc, features, lay, cfg=_DEFAULT_CFG):
    """Per-core bf16+int16 stream, partition-major per 7-slot group."""
    profile, prof_t, fo, iw = lay["profile"], lay["prof_t"], lay["fo"], lay["iw"]
    fb, fw, ib, lw, gb = lay["fb"], lay["fw"], lay["ib"], lay["lw"], lay["gb"]

    blk_perm, win_perm = pc["blk_perm"], pc["win_perm"]
    inv_blk = np.empty(cfg.NBLK, dtype=np.int64)
    inv_blk[blk_perm] = np.arange(cfg.NBLK)
    winrank = np.empty((cfg.NBLK, NW), dtype=np.int64)
    np.put_along_axis(winrank, win_perm,
                      np.broadcast_to(np.arange(NW), (cfg.NBLK, NW)), axis=1)

    bw, u, cnt = pc["bw"], pc["u"], pc["cnt"]
    starts = np.zeros(cfg.NBLK * NW, dtype=np.int64)
    np.cumsum(cnt.ravel()[:-1], out=starts[1:])
    r = np.arange(len(bw), dtype=np.int64) - starts[bw]
    b = bw >> 2
    s = inv_blk[b]
    j = winrank[b, bw & 3]
    i_flat = fo[s, j] + (r >> 7)
    p = r & 127

    hi = features.astype(ml_dtypes.bfloat16).view(np.uint16)

    # slot-major row store: row id = RB[s] + i_flat*128 + p
    rb = np.zeros(cfg.NBLK + 1, dtype=np.int64)
    rb[1:] = np.cumsum(prof_t * BLK)
    rows = np.zeros((int(rb[-1]), F), dtype=np.uint16)
    rows[rb[s] + (i_flat << 7) + p] = hi[pc["idx_e"]]

    # idx store: [prof_t, 128] per slot, bf16 vertex-in-window (pad -> col 0)
    ivb = np.zeros(cfg.NBLK + 1, dtype=np.int64)
    ivb[1:] = np.cumsum(prof_t * BLK)
    ival = np.zeros(int(ivb[-1]), dtype=ml_dtypes.bfloat16)
    ival[ivb[s] + (i_flat << 7) + p] = u.astype(ml_dtypes.bfloat16)
    ival_u = ival.view(np.uint16)

    stream = np.empty(int(gb[-1]), dtype=np.uint16)
    for g in range(cfg.NG):
        vg = stream[int(gb[g]) : int(gb[g + 1])].reshape(BLK, int(lw[g]))
        for q in range(GRP):
            ss = g * GRP + q
            pt = int(prof_t[ss])
            blkrows = rows[rb[ss] : rb[ss + 1]].reshape(pt, BLK, F)
            vg[:, int(fb[g, q]) : int(fb[g, q]) + pt * F] = (
                blkrows.transpose(1, 0, 2).reshape(BLK, pt * F)
            )
            iarr = ival_u[ivb[ss] : ivb[ss + 1]].reshape(pt, BLK).T
            o = int(fw[g]) + int(ib[g, q])
            vg[:, o : o + pt] = iarr
            if int(iw[ss]) > pt:
                vg[:, o + pt : o + int(iw[ss])] = 0  # pad col, never read
    return stream.view(ml_dtypes.bfloat16)


def prep_adjacency(adjacency, pcs, cfg=_DEFAULT_CFG):
    """adj8[c][g, vin, q*T*N + t*N + n] for the permuted vertex at (slot, vin)."""
    adj = np.ascontiguousarray(adjacency.reshape(cfg.V, T, N)).astype(np.int8)
    outs = []
    for c in range(cfg.NCORES):
        apad = np.full((cfg.VPAD, T, N), -1, dtype=np.int8)
        lo = c * cfg.VLOC
        apad[: cfg.VLOC] = adj[lo : lo + cfg.VLOC]
        dev = np.empty((cfg.NBLK, BLK_V, T, N), dtype=np.int8)
        for t in range(T):
            vert = _vert_rows(pcs[t][c], cfg)           # [NBLK, 128]
            dev[:, :, t, :] = apad[vert, t, :]
        dev = dev.reshape(cfg.NG, GRP, BLK_V, T * N).transpose(0, 2, 1, 3)
        outs.append(np.ascontiguousarray(dev).reshape(cfg.NG, BLK, GRP * T * N))
    return outs


def prepare_inputs(adjacency, indices0, features0, indices1, features1,
                   cfg=_DEFAULT_CFG):
    adjacency = np.asarray(adjacency)
    pcs = [shard_table(np.asarray(indices0), cfg),
           shard_table(np.asarray(indices1), cfg)]
    profile = make_profile(pcs, cfg)
    lay = _layout(profile)

    feats = [np.asarray(features0, dtype=np.float32),
             np.asarray(features1, dtype=np.float32)]
    adj8 = prep_adjacency(adjacency, pcs, cfg)
    iota = np.broadcast_to(
        np.tile(np.arange(WIN), lay["tmax"]).astype(ml_dtypes.bfloat16),
        (BLK, lay["tmax"] * WIN),
    ).copy()

    in_maps = []
    for c in range(cfg.NCORES):
        m = {"adj": adj8[c], "iota": iota}
        for t in range(T):
            m[f"feat{t}"] = fill_stream(pcs[t][c], feats[t], lay, cfg)
        in_maps.append(m)
    return in_maps, profile, pcs


def assemble_output(core_outs, pcs, cfg=_DEFAULT_CFG):
    outs = []
    for t in range(T):
        parts = []
        for c in range(cfg.NCORES):
            res = np.asarray(core_outs[c], dtype=np.float32).reshape(
                cfg.NG, BLK, GRP, T, F
            )
            sres = res.transpose(0, 2, 1, 3, 4).reshape(cfg.NBLK, BLK, T, F)
            vert = _vert_rows(pcs[t][c], cfg)
            full = np.empty((cfg.VPAD, F), dtype=np.float32)
            full[vert.ravel()] = sres[:, :, t, :].reshape(-1, F)
            parts.append(full[: cfg.VLOC])
        outs.append(np.concatenate(parts, axis=0).reshape(B, cfg.V, F))
    return (outs[0], outs[1])


def kernel(adjacency, indices0, features0, indices1, features1):
    from concourse.bass_utils import run_bass_kernel_spmd

    cfg = _DEFAULT_CFG
    in_maps, profile, pcs = prepare_inputs(
        adjacency, indices0, features0, indices1, features1, cfg
    )

    key = profile.tobytes()
    if key not in _NC_CACHE:
        _NC_CACHE[key] = build_device_program(profile, cfg)
    nc = _NC_CACHE[key]

    res = run_bass_kernel_spmd(nc, in_maps, list(range(cfg.NCORES)))
    return assemble_output(
        [res.results[c]["out"] for c in range(cfg.NCORES)], pcs, cfg
    )
